# revision 7
# baseline (speedup 1.0000x reference)
"""Trainium2 Bass kernel for nn_ReasoningLayer (per-token MLP with passthrough pos 0).

Full computation:
  out[:, 0]  = hidden_states[:, 0]
  out[:, i]  = GELU(hidden_states[:, i] @ W1 + b1) @ W2 + b2   for i >= 1

Sharding: pure data parallel over batch — core b computes batch b (2048 tokens).
Device works entirely in transposed layout (x^T [D, T]) so both matmuls have the
contraction dim on SBUF partitions with zero on-device transposes:
  h^T = W1^T-stationary @ x^T   (GELU+b1 fused on ScalarE, per-partition bias)
  y^T = W2^T-stationary @ h^T   (b2 added on VectorE, per-partition scalar)
Host casts x/W to bf16 (fp32 PSUM accumulation) and transposes in/out.

v3: the last FP8_JT j-tiles of mm2's contraction run as one fp8e4 DoubleRow
matmul (2 MACs/cell/cycle) into a second PSUM bank:
  - those j-tiles' GELU activations are written by ScalarE directly as fp8e4
    (raw scale: h in [-0.17, ~5] sits in e4m3's normal range),
  - the W2 row-slice is host-quantized with a per-output-column scale
    lambda_o = 224/max|W2_slice[:, o]| (keeps the uniform +-0.044 weights out
    of e4m3 subnormals),
  - VectorE combines: y = (psum_fp8 * (1/lambda_o) + psum_bf16) + b2.
The slice is 1/8 of mm2 -> ~2.7% fewer PE cycles at ~1.4e-2 total rel err
(vs 3.8e-3 pure-bf16, gate 2e-2). Larger slices hit the gate: slice error
scales as 3.75%*sqrt(slice fraction of that matmul) and does NOT dilute
across the other matmul.

v2: y emitted as bf16 (host upconverts; halves output traffic), biases DMA'd
before weights (b1 gates the first activation), W1 j-slice 0 priority chunk,
W2 laid out/DMA'd per output tile in mm2's consumption order.
"""

import numpy as np
import ml_dtypes

B, S, D, H = 8, 2048, 1024, 2048
P = 128
NCORES = 8
TCORE = (B * S) // NCORES  # 2048 tokens per core (== one batch)
TSLAB = 512                # tokens per matmul moving-operand slab
NSLAB = TCORE // TSLAB     # 4
DO = D // P                # 8  k-tiles for matmul 1
JO = H // P                # 16 j-tiles (hidden)
OO = D // P                # 8  output tiles

COMPUTE = "bf16"           # "bf16" or "fp32r"
FP8_JT = 4                 # max trailing j-tiles of mm2 in fp8 DoubleRow (0 = off)
FP8_JT_BY_SLAB = (4, 4, 2, 2)  # per-slab fp8 j-tiles: 18.75% of mm2 in fp8

_nc_cache = {}


def _build(compute, repeat=1, tslab=None, fp8_jt=FP8_JT):
    import concourse.bass as bass
    import concourse.mybir as mybir
    import concourse.tile as tile
    from concourse import bacc

    f32 = mybir.dt.float32
    bf16 = mybir.dt.bfloat16
    e4 = mybir.dt.float8e4
    cdt = bf16 if compute == "bf16" else mybir.dt.float32r
    if compute != "bf16":
        fp8_jt = 0
    if tslab is None:
        tslab = 512 if compute == "bf16" else 256
    TSLAB = tslab
    NSLAB = TCORE // TSLAB
    ts = bass.ts
    Gelu = mybir.ActivationFunctionType.Gelu
    DR = mybir.MatmulPerfMode.DoubleRow
    Alu = mybir.AluOpType
    jt_by_slab = FP8_JT_BY_SLAB if fp8_jt else (0,) * NSLAB
    JBMIN = JO - fp8_jt    # smallest bf16 j-tile count

    nc = bacc.Bacc("TRN2", target_bir_lowering=False, debug=False,
                   num_devices=NCORES)
    xT = nc.dram_tensor("xT", [D, TCORE], cdt, kind="ExternalInput")
    w1 = nc.dram_tensor("w1", [D, H], cdt, kind="ExternalInput")
    b1 = nc.dram_tensor("b1", [H], f32, kind="ExternalInput")
    w2 = nc.dram_tensor("w2", [H, D], cdt, kind="ExternalInput")
    b2 = nc.dram_tensor("b2", [D], f32, kind="ExternalInput")
    if fp8_jt:
        w2f8 = nc.dram_tensor("w2f8", [P, fp8_jt, OO, P], e4,
                              kind="ExternalInput")
        l2i = nc.dram_tensor("l2i", [D], f32, kind="ExternalInput")
    yT = nc.dram_tensor("yT", [D, TCORE], cdt, kind="ExternalOutput")

    with tile.TileContext(nc) as tc:
        with (
            tc.tile_pool(name="w", bufs=1) as wpool,
            tc.tile_pool(name="bias", bufs=1) as bpool,
            tc.tile_pool(name="x", bufs=2) as xpool,
            tc.tile_pool(name="h", bufs=2) as hpool,
            tc.tile_pool(name="y", bufs=2) as ypool,
            tc.tile_pool(name="t", bufs=2) as tpool,
            tc.tile_pool(name="ps1", bufs=4, space=bass.MemorySpace.PSUM) as pp1,
            tc.tile_pool(name="ps2", bufs=2 if FP8_JT else 4,
                         space=bass.MemorySpace.PSUM) as pp2,
        ):
            # Biases first: b1 gates the first activation (~2us in). 12KB,
            # must not queue behind 8MB of weights.
            b1_sb = bpool.tile([P, JO], f32, name="b1_sb")
            nc.sync.dma_start(b1_sb[:], b1.rearrange("(jo ji) -> ji jo", ji=P))
            b2_sb = bpool.tile([P, OO], f32, name="b2_sb")
            nc.sync.dma_start(b2_sb[:], b2.rearrange("(oo oi) -> oi oo", oi=P))
            if fp8_jt:
                l2i_sb = bpool.tile([P, OO], f32, name="l2i_sb")
                nc.sync.dma_start(l2i_sb[:],
                                  l2i.rearrange("(oo oi) -> oi oo", oi=P))
                w2f8_sb = wpool.tile([P, fp8_jt, OO, P], e4, name="w2f8_sb")
                nc.sync.dma_start(w2f8_sb[:], w2f8[:])

            # W1, j-consumption order; small priority chunk covers the very
            # first jt-group so the PE starts immediately.
            w1_sb = wpool.tile([P, DO, H], cdt, name="w1_sb")
            w1r = w1.rearrange("(do di) j -> di do j", di=P)
            for do in range(DO):
                nc.sync.dma_start(w1_sb[:, do, ts(0, P)], w1r[:, do, ts(0, P)])
            for jh in range(4):
                lo = P if jh == 0 else 0
                for do in range(DO):
                    nc.sync.dma_start(
                        w1_sb[:, do, jh * (H // 4) + lo: (jh + 1) * (H // 4)],
                        w1r[:, do, jh * (H // 4) + lo: (jh + 1) * (H // 4)],
                    )
            # W2 bf16 part [ji, ot, jo<JB, oi], DMA'd in mm2's ot-major
            # consumption order.
            JBMAX = JO - (min(jt_by_slab) if fp8_jt else 0)
            w2_sb = wpool.tile([P, OO, JBMAX, P], cdt, name="w2_sb")
            w2r = w2.rearrange("(jo ji) (oo oi) -> ji oo jo oi", ji=P, oi=P)
            for ot in range(OO):
                for jh in range(2):
                    nc.sync.dma_start(
                        w2_sb[:, ot, ts(jh, JBMAX // 2)],
                        w2r[:, ot, ts(jh, JBMAX // 2)],
                    )

            xTr = xT.rearrange("(do di) t -> di do t", di=P)
            yTr = yT.rearrange("(oo oi) t -> oi oo t", oi=P)

            for it in [i for _ in range(repeat) for i in range(NSLAB)]:
                njt8 = jt_by_slab[it]       # fp8 j-tiles this slab
                JB = JO - njt8
                x_sb = xpool.tile([P, DO, TSLAB], cdt, tag="x_sb")
                for do in range(DO):
                    nc.sync.dma_start(x_sb[:, do], xTr[:, do, ts(it, TSLAB)])

                h_sb = hpool.tile([P, JO - min(jt_by_slab) if fp8_jt else JO,
                                   TSLAB], cdt, tag="h_sb")
                if fp8_jt:
                    hf8 = hpool.tile([P, fp8_jt, TSLAB], e4, tag="hf8")
                for jt in range(JO):
                    ps = pp1.tile([P, TSLAB], f32, tag="ps1")
                    for kt in range(DO):
                        nc.tensor.matmul(
                            ps[:],
                            w1_sb[:, kt, ts(jt, P)],
                            x_sb[:, kt],
                            start=(kt == 0),
                            stop=(kt == DO - 1),
                        )
                    # h^T[j_tile] = Gelu(psum + b1[j_tile]); trailing tiles go
                    # straight to fp8e4 for the DoubleRow matmul. fp8 slot i
                    # always maps to j-tile JBMIN+i so W2f8 slots line up.
                    dst = h_sb[:, jt] if jt < JB else hf8[:, jt - JBMIN]
                    nc.scalar.activation(dst, ps[:], Gelu,
                                         bias=b1_sb[:, ts(jt, 1)])

                y_sb = ypool.tile([P, OO, TSLAB], cdt, tag="y_sb")
                for ot in range(OO):
                    ps2 = pp2.tile([P, TSLAB], f32, tag="ps2")
                    for jt in range(JB):
                        nc.tensor.matmul(
                            ps2[:],
                            w2_sb[:, ot, jt],
                            h_sb[:, jt],
                            start=(jt == 0),
                            stop=(jt == JB - 1),
                        )
                    if fp8_jt:
                        ps2b = pp2.tile([P, TSLAB], f32, tag="ps2b")
                        pairs = range((JB - JBMIN) // 2, fp8_jt // 2)
                        for n, g in enumerate(pairs):
                            nc.tensor.matmul(
                                ps2b[:],
                                w2f8_sb[:, 2 * g:2 * g + 2, ot, :],
                                hf8[:, 2 * g:2 * g + 2],
                                start=(n == 0),
                                stop=(n == len(pairs) - 1),
                                perf_mode=DR,
                            )
                        t_sb = tpool.tile([P, TSLAB], f32, tag="t_sb")
                        # DVE may read only one PSUM operand per op:
                        # t = psum_fp8 / lambda_o ; y = (t + b2) + psum_bf16
                        nc.vector.tensor_scalar_mul(t_sb[:], ps2b[:],
                                                    l2i_sb[:, ts(ot, 1)])
                        nc.vector.scalar_tensor_tensor(
                            y_sb[:, ot], t_sb[:], b2_sb[:, ts(ot, 1)], ps2[:],
                            Alu.add, Alu.add,
                        )
                    else:
                        nc.vector.tensor_scalar_add(y_sb[:, ot], ps2[:],
                                                    b2_sb[:, ts(ot, 1)])
                for oo in range(OO):
                    nc.sync.dma_start(yTr[:, oo, ts(it, TSLAB)], y_sb[:, oo])

    nc.compile()
    return nc


def _get_nc(compute=COMPUTE, repeat=1, tslab=None):
    key = (compute, repeat, tslab)
    if key not in _nc_cache:
        _nc_cache[key] = _build(compute, repeat, tslab)
    return _nc_cache[key]


def _quant_inputs(W2, compute):
    """Host-side fp8 quantization of the trailing FP8_JT j-tiles of W2."""
    if FP8_JT == 0 or compute != "bf16":
        return None, None
    e4np = ml_dtypes.float8_e4m3
    W2s = np.asarray(W2, np.float32)[(JO - FP8_JT) * P:, :]   # [FP8_JT*P, D]
    lam = 224.0 / np.maximum(np.max(np.abs(W2s), axis=0), 1e-30)  # [D]
    w2q = (W2s * lam[None, :]).astype(e4np)                   # [FP8_JT*P, D]
    # -> [P, fp8_jt, OO, P]: w2f8[p, i, ot, oi] = q(W2[(JB+i)*P + p, ot*P+oi])
    w2f8 = np.ascontiguousarray(
        w2q.reshape(FP8_JT, P, OO, P).transpose(1, 0, 2, 3))
    l2i = np.ascontiguousarray((1.0 / lam).astype(np.float32))
    return w2f8, l2i


def _run(hidden_states, W1, b1, W2, b2, compute=COMPUTE, trace=False):
    from concourse import bass_utils

    nc = _get_nc(compute)
    hidden_states = np.asarray(hidden_states, np.float32)
    cnp = ml_dtypes.bfloat16 if compute == "bf16" else np.float32
    W1c = np.ascontiguousarray(np.asarray(W1).astype(cnp))
    W2c = np.ascontiguousarray(np.asarray(W2).astype(cnp))
    b1c = np.ascontiguousarray(np.asarray(b1, np.float32))
    b2c = np.ascontiguousarray(np.asarray(b2, np.float32))
    w2f8, l2i = _quant_inputs(W2, compute)

    in_maps = []
    for c in range(NCORES):
        # order='C' is load-bearing: .T is a strided view and astype's default
        # order='K' would keep it F-ordered, which binds wrong bytes on the
        # native NRT path.
        xT_c = hidden_states[c].T.astype(cnp, order="C")  # [D, TCORE]
        m = {"xT": xT_c, "w1": W1c, "b1": b1c, "w2": W2c, "b2": b2c}
        if w2f8 is not None:
            m["w2f8"] = w2f8
            m["l2i"] = l2i
        in_maps.append(m)

    res = bass_utils.run_bass_kernel_spmd(
        nc, in_maps, core_ids=list(range(NCORES)), trace=trace
    )

    out = np.empty((B, S, D), np.float32)
    for c in range(NCORES):
        out[c] = res.results[c]["yT"].astype(np.float32).T
    out[:, 0, :] = hidden_states[:, 0, :]
    return out, res


def _bench_in_maps(compute=COMPUTE):
    """Random full-shape inputs for timing runs (bench_ab)."""
    rng = np.random.default_rng(0)
    cnp = ml_dtypes.bfloat16 if compute == "bf16" else np.float32
    xT = rng.standard_normal((D, TCORE)).astype(cnp)
    w1 = (rng.standard_normal((D, H)) * 0.03).astype(cnp)
    w2 = (rng.standard_normal((H, D)) * 0.03).astype(cnp)
    b1v = np.zeros(H, np.float32)
    b2v = np.zeros(D, np.float32)
    m = {"xT": xT, "w1": w1, "b1": b1v, "w2": w2, "b2": b2v}
    w2f8, l2i = _quant_inputs(w2.astype(np.float32), compute)
    if w2f8 is not None:
        m["w2f8"] = w2f8
        m["l2i"] = l2i
    return [dict(m) for _ in range(NCORES)]


def kernel(hidden_states, W1, b1, W2, b2):
    out, _ = _run(hidden_states, W1, b1, W2, b2)
    return out


# revision 13
# speedup vs baseline: 1.0047x; 1.0047x over previous
"""Trainium2 Bass kernel for nn_ReasoningLayer (per-token MLP with passthrough pos 0).

Full computation:
  out[:, 0]  = hidden_states[:, 0]
  out[:, i]  = GELU(hidden_states[:, i] @ W1 + b1) @ W2 + b2   for i >= 1

Sharding: pure data parallel over batch — core b computes batch b (2048 tokens).
Device works entirely in transposed layout (x^T [D, T]) so both matmuls have the
contraction dim on SBUF partitions with zero on-device transposes:
  h^T = W1^T-stationary @ x^T   (GELU+b1 fused on ScalarE, per-partition bias)
  y^T = W2^T-stationary @ h^T   (b2 added on VectorE, per-partition scalar)
Host casts x/W to bf16 (fp32 PSUM accumulation) and transposes in/out.

v3/v4: the trailing FP8_JT_BY_SLAB[slab] j-tiles of mm2's contraction run
as fp8e4 DoubleRow matmuls (2 MACs/cell/cycle, pairs of j-tiles per MM) into
a second PSUM bank:
  - those j-tiles' GELU activations are written by ScalarE directly as fp8e4
    (raw scale: h in [-0.17, ~5] sits in e4m3's normal range),
  - the W2 row-slice is host-quantized with a per-output-column scale
    lambda_o = 224/max|W2_slice[:, o]| (keeps the uniform +-0.044 weights out
    of e4m3 subnormals),
  - VectorE combines: y = (psum_fp8 * (1/lambda_o) + psum_bf16) + b2.
Config (2,2,4,4) puts 18.75% of mm2 in fp8 (heavy slabs last to shrink
the serial tail) -> 4.1% fewer PE cycles at 1.630e-2 measured rel err (pure-bf16 floor 3.8e-3, gate 2e-2; deterministic
across runs since inputs and instruction stream are fixed). The error scales
as 3.65%*sqrt(slice fraction OF mm2) and does NOT dilute across mm1 — a
uniform (4,4,4,4) slice measured 1.87e-2, too close to the gate to ship; mm1
cannot participate (its minimum DR slice is 1/4 of its K -> ~1.9% alone).

v2: y emitted as bf16 (host upconverts; halves output traffic), biases DMA'd
before weights (b1 gates the first activation), W1 j-slice 0 priority chunk,
W2 laid out/DMA'd per output tile in mm2's consumption order.
"""

import numpy as np
import ml_dtypes

B, S, D, H = 8, 2048, 1024, 2048
P = 128
NCORES = 8
TCORE = (B * S) // NCORES  # 2048 tokens per core (== one batch)
TSLAB = 512                # tokens per matmul moving-operand slab
NSLAB = TCORE // TSLAB     # 4
DO = D // P                # 8  k-tiles for matmul 1
JO = H // P                # 16 j-tiles (hidden)
OO = D // P                # 8  output tiles

COMPUTE = "bf16"           # "bf16" or "fp32r"
FP8_JT = 4                 # max trailing j-tiles of mm2 in fp8 DoubleRow (0 = off)
FP8_JT_BY_SLAB = (2, 2, 4, 4)  # per-slab fp8 j-tiles: 18.75% of mm2 in fp8;
                               # fp8-heavy slabs last so the serial tail
                               # (final ot-group) is the cheaper variant

_nc_cache = {}


def _build(compute, repeat=1, tslab=None, fp8_jt=FP8_JT):
    import concourse.bass as bass
    import concourse.mybir as mybir
    import concourse.tile as tile
    from concourse import bacc

    f32 = mybir.dt.float32
    bf16 = mybir.dt.bfloat16
    e4 = mybir.dt.float8e4
    cdt = bf16 if compute == "bf16" else mybir.dt.float32r
    if compute != "bf16":
        fp8_jt = 0
    if tslab is None:
        tslab = 512 if compute == "bf16" else 256
    TSLAB = tslab
    NSLAB = TCORE // TSLAB
    ts = bass.ts
    Gelu = mybir.ActivationFunctionType.Gelu
    DR = mybir.MatmulPerfMode.DoubleRow
    Alu = mybir.AluOpType
    jt_by_slab = FP8_JT_BY_SLAB if fp8_jt else (0,) * NSLAB
    JBMIN = JO - fp8_jt    # smallest bf16 j-tile count

    nc = bacc.Bacc("TRN2", target_bir_lowering=False, debug=False,
                   num_devices=NCORES)
    xT = nc.dram_tensor("xT", [D, TCORE], cdt, kind="ExternalInput")
    w1 = nc.dram_tensor("w1", [D, H], cdt, kind="ExternalInput")
    b1 = nc.dram_tensor("b1", [H], f32, kind="ExternalInput")
    w2 = nc.dram_tensor("w2", [H, D], cdt, kind="ExternalInput")
    b2 = nc.dram_tensor("b2", [D], f32, kind="ExternalInput")
    if fp8_jt:
        w2f8 = nc.dram_tensor("w2f8", [P, fp8_jt, OO, P], e4,
                              kind="ExternalInput")
        l2i = nc.dram_tensor("l2i", [D], f32, kind="ExternalInput")
    yT = nc.dram_tensor("yT", [D, TCORE], cdt, kind="ExternalOutput")

    with tile.TileContext(nc) as tc:
        with (
            tc.tile_pool(name="w", bufs=1) as wpool,
            tc.tile_pool(name="bias", bufs=1) as bpool,
            tc.tile_pool(name="x", bufs=2) as xpool,
            tc.tile_pool(name="h", bufs=2) as hpool,
            tc.tile_pool(name="y", bufs=2) as ypool,
            tc.tile_pool(name="t", bufs=2) as tpool,
            tc.tile_pool(name="ps1", bufs=4, space=bass.MemorySpace.PSUM) as pp1,
            tc.tile_pool(name="ps2", bufs=2 if FP8_JT else 4,
                         space=bass.MemorySpace.PSUM) as pp2,
        ):
            # Load the Gelu table while the DMA front runs (first real
            # activation fires ~2us in and would otherwise eat the table
            # load latency).
            warm = bpool.tile([P, 2], f32, name="warm")
            nc.vector.memset(warm[:, 0:1], 0.0)
            nc.scalar.activation(warm[:, 1:2], warm[:, 0:1], Gelu)

            # Weights + biases go on the Activation engine's HWDGE ring;
            # x/y streams use the SP ring. HWDGE DMAs are FIFO per issuing
            # engine, so this keeps the first x slab from queueing behind
            # ~8MB of weights. Biases first: b1 gates the first activation.
            b1_sb = bpool.tile([P, JO], f32, name="b1_sb")
            nc.scalar.dma_start(b1_sb[:], b1.rearrange("(jo ji) -> ji jo", ji=P))
            b2_sb = bpool.tile([P, OO], f32, name="b2_sb")
            nc.scalar.dma_start(b2_sb[:], b2.rearrange("(oo oi) -> oi oo", oi=P))

            # W1 priority chunk right after the biases: it gates the very
            # first matmul. w2f8/l2i aren't consumed until mm2 (~25us in),
            # so they queue after it.
            w1_sb = wpool.tile([P, DO, H], cdt, name="w1_sb")
            w1r = w1.rearrange("(do di) j -> di do j", di=P)
            nc.scalar.dma_start(w1_sb[:, :, ts(0, P)], w1r[:, :, ts(0, P)])
            if fp8_jt:
                l2i_sb = bpool.tile([P, OO], f32, name="l2i_sb")
                nc.scalar.dma_start(l2i_sb[:],
                                    l2i.rearrange("(oo oi) -> oi oo", oi=P))
                w2f8_sb = wpool.tile([P, fp8_jt, OO, P], e4, name="w2f8_sb")
                nc.scalar.dma_start(w2f8_sb[:], w2f8[:])
            for jh in range(4):
                lo = P if jh == 0 else 0
                nc.scalar.dma_start(
                    w1_sb[:, :, jh * (H // 4) + lo: (jh + 1) * (H // 4)],
                    w1r[:, :, jh * (H // 4) + lo: (jh + 1) * (H // 4)],
                )
            # W2 bf16 part [ji, ot, jo<JB, oi], DMA'd in mm2's ot-major
            # consumption order.
            JBMAX = JO - (min(jt_by_slab) if fp8_jt else 0)
            w2_sb = wpool.tile([P, OO, JBMAX, P], cdt, name="w2_sb")
            w2r = w2.rearrange("(jo ji) (oo oi) -> ji oo jo oi", ji=P, oi=P)
            for ot in range(OO):
                nc.scalar.dma_start(w2_sb[:, ot], w2r[:, ot, 0:JBMAX])

            xTr = xT.rearrange("(do di) t -> di do t", di=P)
            yTr = yT.rearrange("(oo oi) t -> oi oo t", oi=P)

            for it in [i for _ in range(repeat) for i in range(NSLAB)]:
                njt8 = jt_by_slab[it]       # fp8 j-tiles this slab
                JB = JO - njt8
                x_sb = xpool.tile([P, DO, TSLAB], cdt, tag="x_sb")
                for do in range(DO):
                    nc.sync.dma_start(x_sb[:, do], xTr[:, do, ts(it, TSLAB)])

                h_sb = hpool.tile([P, JO - min(jt_by_slab) if fp8_jt else JO,
                                   TSLAB], cdt, tag="h_sb")
                if fp8_jt:
                    hf8 = hpool.tile([P, fp8_jt, TSLAB], e4, tag="hf8")
                for jt in range(JO):
                    ps = pp1.tile([P, TSLAB], f32, tag="ps1")
                    for kt in range(DO):
                        nc.tensor.matmul(
                            ps[:],
                            w1_sb[:, kt, ts(jt, P)],
                            x_sb[:, kt],
                            start=(kt == 0),
                            stop=(kt == DO - 1),
                        )
                    # h^T[j_tile] = Gelu(psum + b1[j_tile]); trailing tiles go
                    # straight to fp8e4 for the DoubleRow matmul. fp8 slot i
                    # always maps to j-tile JBMIN+i so W2f8 slots line up.
                    dst = h_sb[:, jt] if jt < JB else hf8[:, jt - JBMIN]
                    nc.scalar.activation(dst, ps[:], Gelu,
                                         bias=b1_sb[:, ts(jt, 1)])

                y_sb = ypool.tile([P, OO, TSLAB], cdt, tag="y_sb")
                for ot in range(OO):
                    ps2 = pp2.tile([P, TSLAB], f32, tag="ps2")
                    for jt in range(JB):
                        nc.tensor.matmul(
                            ps2[:],
                            w2_sb[:, ot, jt],
                            h_sb[:, jt],
                            start=(jt == 0),
                            stop=(jt == JB - 1),
                        )
                    if fp8_jt:
                        ps2b = pp2.tile([P, TSLAB], f32, tag="ps2b")
                        pairs = range((JB - JBMIN) // 2, fp8_jt // 2)
                        for n, g in enumerate(pairs):
                            nc.tensor.matmul(
                                ps2b[:],
                                w2f8_sb[:, 2 * g:2 * g + 2, ot, :],
                                hf8[:, 2 * g:2 * g + 2],
                                start=(n == 0),
                                stop=(n == len(pairs) - 1),
                                perf_mode=DR,
                            )
                        t_sb = tpool.tile([P, TSLAB], f32, tag="t_sb")
                        # DVE may read only one PSUM operand per op:
                        # t = psum_fp8 / lambda_o ; y = (t + b2) + psum_bf16
                        nc.vector.tensor_scalar_mul(t_sb[:], ps2b[:],
                                                    l2i_sb[:, ts(ot, 1)])
                        nc.vector.scalar_tensor_tensor(
                            y_sb[:, ot], t_sb[:], b2_sb[:, ts(ot, 1)], ps2[:],
                            Alu.add, Alu.add,
                        )
                    else:
                        nc.vector.tensor_scalar_add(y_sb[:, ot], ps2[:],
                                                    b2_sb[:, ts(ot, 1)])
                for oo in range(OO):
                    nc.sync.dma_start(yTr[:, oo, ts(it, TSLAB)], y_sb[:, oo])

    nc.compile()
    return nc


def _get_nc(compute=COMPUTE, repeat=1, tslab=None):
    key = (compute, repeat, tslab)
    if key not in _nc_cache:
        _nc_cache[key] = _build(compute, repeat, tslab)
    return _nc_cache[key]


def _quant_inputs(W2, compute):
    """Host-side fp8 quantization of the trailing FP8_JT j-tiles of W2."""
    if FP8_JT == 0 or compute != "bf16":
        return None, None
    e4np = ml_dtypes.float8_e4m3
    W2s = np.asarray(W2, np.float32)[(JO - FP8_JT) * P:, :]   # [FP8_JT*P, D]
    lam = 224.0 / np.maximum(np.max(np.abs(W2s), axis=0), 1e-30)  # [D]
    w2q = (W2s * lam[None, :]).astype(e4np)                   # [FP8_JT*P, D]
    # -> [P, fp8_jt, OO, P]: w2f8[p, i, ot, oi] = q(W2[(JB+i)*P + p, ot*P+oi])
    w2f8 = np.ascontiguousarray(
        w2q.reshape(FP8_JT, P, OO, P).transpose(1, 0, 2, 3))
    l2i = np.ascontiguousarray((1.0 / lam).astype(np.float32))
    return w2f8, l2i


def _run(hidden_states, W1, b1, W2, b2, compute=COMPUTE, trace=False):
    from concourse import bass_utils

    nc = _get_nc(compute)
    hidden_states = np.asarray(hidden_states, np.float32)
    cnp = ml_dtypes.bfloat16 if compute == "bf16" else np.float32
    W1c = np.ascontiguousarray(np.asarray(W1).astype(cnp))
    W2c = np.ascontiguousarray(np.asarray(W2).astype(cnp))
    b1c = np.ascontiguousarray(np.asarray(b1, np.float32))
    b2c = np.ascontiguousarray(np.asarray(b2, np.float32))
    w2f8, l2i = _quant_inputs(W2, compute)

    in_maps = []
    for c in range(NCORES):
        # order='C' is load-bearing: .T is a strided view and astype's default
        # order='K' would keep it F-ordered, which binds wrong bytes on the
        # native NRT path.
        xT_c = hidden_states[c].T.astype(cnp, order="C")  # [D, TCORE]
        m = {"xT": xT_c, "w1": W1c, "b1": b1c, "w2": W2c, "b2": b2c}
        if w2f8 is not None:
            m["w2f8"] = w2f8
            m["l2i"] = l2i
        in_maps.append(m)

    res = bass_utils.run_bass_kernel_spmd(
        nc, in_maps, core_ids=list(range(NCORES)), trace=trace
    )

    out = np.empty((B, S, D), np.float32)
    for c in range(NCORES):
        out[c] = res.results[c]["yT"].astype(np.float32).T
    out[:, 0, :] = hidden_states[:, 0, :]
    return out, res


def _bench_in_maps(compute=COMPUTE):
    """Random full-shape inputs for timing runs (bench_ab)."""
    rng = np.random.default_rng(0)
    cnp = ml_dtypes.bfloat16 if compute == "bf16" else np.float32
    xT = rng.standard_normal((D, TCORE)).astype(cnp)
    w1 = (rng.standard_normal((D, H)) * 0.03).astype(cnp)
    w2 = (rng.standard_normal((H, D)) * 0.03).astype(cnp)
    b1v = np.zeros(H, np.float32)
    b2v = np.zeros(D, np.float32)
    m = {"xT": xT, "w1": w1, "b1": b1v, "w2": w2, "b2": b2v}
    w2f8, l2i = _quant_inputs(w2.astype(np.float32), compute)
    if w2f8 is not None:
        m["w2f8"] = w2f8
        m["l2i"] = l2i
    return [dict(m) for _ in range(NCORES)]


def kernel(hidden_states, W1, b1, W2, b2):
    out, _ = _run(hidden_states, W1, b1, W2, b2)
    return out
